# revision 26
# baseline (speedup 1.0000x reference)
"""CharRNN Trainium2 kernel: logits, final_hidden = rnn(x, embedding, ...).

Strategy (8 NeuronCores, data-parallel over batch, 32 rows/core):
  - Host folds embedding @ W_ih + b_h into a [VOCAB, HIDDEN] table and
    lowers x to a one-hot indicator; the input projection xp_t enters
    each step's PSUM accumulation via a one-hot matmul (start=True).
  - Hidden state lives transposed [H, B] in SBUF (bf16).
  - Port-split recurrence: the PE has independent SBUF read ports for
    the stationary (weights) and moving operands. H_out blocks
    m < SPLIT are computed weight-stationary (W_hh blocks via the
    FWL weight port, h^T moving, N=32); blocks m >= SPLIT are computed
    h-stationary (h^T chunks on the weight port, W_hh rows streamed
    512-wide on the moving port). The two streams share the array but
    load W through different ports, beating the single-port W-movement
    floor. The h-stationary half also keeps the PE HAM-warm (dense
    512-col matmuls at 2.4 GHz).
  - The h-stationary half lands batch-major; DVE 32x32 StreamTranspose
    puts it back into h^T form (PE untouched).
  - Every 16 steps the banked h^T history feeds the W_ho projection
    (N=512 matmuls), bias added on DVE, result DMA'd out transposed.
"""

import numpy as np
import ml_dtypes

import concourse.bass as bass
import concourse.mybir as mybir
import concourse.tile as tile
from concourse.bass_utils import run_bass_kernel_spmd

N_CORES = 8
V = 64        # vocab
E = 256       # embed
H = 1024      # hidden
HB = H // 128  # 8 H-blocks
CHUNK = 16    # steps per logits flush (16*32 = 512 moving cols)
SPLIT = 4     # H-blocks computed weight-stationary; rest h-stationary

BF16 = mybir.dt.bfloat16
F32 = mybir.dt.float32

TRACE = False           # set True from test harness to get NTFF profile
LAST_RESULT = {}        # exec_time_ns etc. stashed here for the harness


def _split_excess_waits(nc):
    """This walrus build encodes at most 1 sync wait per instruction
    (2 for EventSemaphore). Tile's tail drain can carry more; hoist the
    excess into preceding NOPs on the same engine."""
    ctr = 0
    for f in nc.m.functions:
        for bb in f.blocks:
            new_list = []
            changed = False
            for inst in bb.instructions:
                si = inst.sync_info
                cap = 2 if isinstance(inst, mybir.InstEventSemaphore) else 1
                if si is not None and si.on_wait and len(si.on_wait) > cap:
                    waits = list(si.on_wait)
                    keep, excess = waits[:cap], waits[cap:]
                    for w in excess:
                        new_list.append(mybir.InstNoOp(
                            name=f"wait_split_{ctr}",
                            engine=inst.engine,
                            bass_nofuse=True,
                            sync_info=mybir.SyncInfo(on_wait=[w], on_update=[]),
                        ))
                        ctr += 1
                    si.on_wait = keep
                    inst.sync_info = si
                    changed = True
                new_list.append(inst)
            if changed:
                bb.instructions = new_list
    return ctr


def _build_nc(L, BL):
    NB = (HB - SPLIT) * 128      # h-stationary H_out columns (<= 512)
    assert NB <= 512
    nc = bass.Bass()
    ws_d = nc.dram_tensor("w_stat", [128, SPLIT * HB * 128], BF16,
                          kind="ExternalInput")
    wm_d = nc.dram_tensor("w_mov", [128, HB * NB], BF16, kind="ExternalInput")
    tab_d = nc.dram_tensor("table", [V, H], BF16, kind="ExternalInput")
    oh_d = nc.dram_tensor("onehot", [V, L * BL], BF16, kind="ExternalInput")
    who_d = nc.dram_tensor("w_ho", [128, HB * V], BF16, kind="ExternalInput")
    bo_d = nc.dram_tensor("b_o", [V, 1], F32, kind="ExternalInput")
    log_d = nc.dram_tensor("logits_t", [V, L * BL], F32, kind="ExternalOutput")
    hfa_d = nc.dram_tensor("h_final_a", [128, SPLIT * BL], F32,
                           kind="ExternalOutput")
    hfb_d = nc.dram_tensor("h_final_b", [BL, NB], F32, kind="ExternalOutput")

    Tanh = mybir.ActivationFunctionType.Tanh
    SQ = CHUNK * BL          # 512: cols per hist k-block

    with tile.TileContext(nc) as tc:
        with (
            tc.tile_pool(name="const", bufs=1) as cpool,
            tc.tile_pool(name="hist", bufs=1) as hpool,
            tc.tile_pool(name="stage", bufs=3) as spool,
            tc.tile_pool(name="hb", bufs=2) as hbpool,
            tc.tile_pool(name="psa", bufs=4, space="PSUM") as psa_pool,
            tc.tile_pool(name="psb", bufs=2, space="PSUM") as psb_pool,
            tc.tile_pool(name="psl", bufs=2, space="PSUM") as psl_pool,
        ):
            ws_sb = cpool.tile([128, SPLIT * HB * 128], BF16, tag="ws")
            wm_sb = cpool.tile([128, HB * NB], BF16, tag="wm")
            tab_sb = cpool.tile([V, H], BF16, tag="tab")
            oh_sb = cpool.tile([V, L * BL], BF16, tag="oh")
            who_sb = cpool.tile([128, HB * V], BF16, tag="who")
            bo_sb = cpool.tile([V, 1], F32, tag="bo")
            hfa_sb = cpool.tile([128, SPLIT * BL], F32, tag="hfa")
            # order matters: step 0 gates only on the table and the first
            # few steps' onehot columns; the 2MB weight loads (not needed
            # until t=1) and the onehot tail go behind them
            OH0 = min(8, L) * BL
            nc.sync.dma_start(tab_sb[:], tab_d[:])
            nc.sync.dma_start(oh_sb[:, :OH0], oh_d[:, :OH0])
            nc.sync.dma_start(bo_sb[:], bo_d[:])
            # wm before ws: each step runs group B (wm) before group A (ws)
            nc.sync.dma_start(wm_sb[:], wm_d[:])
            nc.sync.dma_start(ws_sb[:], ws_d[:])
            if OH0 < L * BL:
                nc.sync.dma_start(oh_sb[:, OH0:], oh_d[:, OH0:])
            nc.sync.dma_start(who_sb[:], who_d[:])

            # h^T history, double-buffered by 16-step chunk parity:
            # hist[par][:, k*SQ + slot*BL + b] = h_t[k*128+p, b], slot=t%16
            hist = [hpool.tile([128, HB * SQ], BF16, tag=f"h{p}",
                               name=f"hist{p}") for p in range(2)]
            hist3 = [h.rearrange("p (k s) -> p k s", k=HB) for h in hist]

            for t in range(L):
                par, slot = (t // CHUNK) % 2, t % CHUNK
                ppar, pslot = ((t - 1) // CHUNK) % 2, (t - 1) % CHUNK
                oh_t = oh_sb[:, t * BL:(t + 1) * BL]

                # ---- group B: h-stationary, W streamed on moving port ----
                psb = psb_pool.tile([BL, NB], F32)
                # xp (batch-major): onehot_t stationary, table slice moving
                nc.tensor.matmul(
                    psb[:], oh_t, tab_sb[:, SPLIT * 128:],
                    start=True, stop=(t == 0))
                if t > 0:
                    for k in range(HB):
                        nc.tensor.matmul(
                            psb[:],
                            hist3[ppar][:, k, pslot * BL:(pslot + 1) * BL],
                            wm_sb[:, k * NB:(k + 1) * NB],
                            start=False, stop=(k == HB - 1),
                        )

                # ---- group A: weight-stationary W blocks, h^T moving ----
                # per-pair PSUM tiles (separate banks) so the early tanh of
                # one pair never bank-conflicts with matmuls of the next
                psa_tiles = []
                for pr in range(SPLIT // 2):
                    psa = psa_pool.tile([128, 2 * BL], F32)
                    psa_tiles.append(psa)
                    for mm in range(2):
                        m = 2 * pr + mm
                        reg = psa[:, mm * BL:(mm + 1) * BL]
                        nc.tensor.matmul(
                            reg, tab_sb[:, m * 128:(m + 1) * 128], oh_t,
                            start=True, stop=(t == 0))
                        if t > 0:
                            for k in range(HB):
                                nc.tensor.matmul(
                                    reg,
                                    ws_sb[:, (m * HB + k) * 128:(m * HB + k + 1) * 128],
                                    hist3[ppar][:, k, pslot * BL:(pslot + 1) * BL],
                                    start=False, stop=(k == HB - 1),
                                )
                    # early tanh per pair of A-blocks so next step's group B
                    # (k-order 0..7) finds its stationaries ready
                    nc.scalar.activation(
                        hist3[par][:, 2 * pr:2 * pr + 2, slot * BL:(slot + 1) * BL],
                        psa.rearrange("p (k s) -> p k s", k=2),
                        Tanh)

                # ---- group B epilogue: tanh + DVE 32x32 transpose ----
                # hb columns are host-permuted q-major: jj = q*128+k'*32+s
                # holds H column SPLIT*128 + k'*128 + q*32 + s, so each
                # contiguous 128-col slice transposes into one partition
                # sub-range across all 4 k-blocks (2D-in + 3D-out form).
                hb = hbpool.tile([BL, NB], BF16, tag="hb")
                nc.scalar.activation(hb[:], psb[:], Tanh)
                for q in range(4):
                    nc.vector.transpose(
                        hist3[par][q * 32:(q + 1) * 32, SPLIT:, slot * BL:(slot + 1) * BL],
                        hb[:, q * 128:(q + 1) * 128],
                    )

                if t == L - 1:
                    for pr in range(SPLIT // 2):
                        nc.scalar.activation(
                            hfa_sb[:, 2 * pr * BL:(2 * pr + 2) * BL],
                            psa_tiles[pr][:], Tanh)
                    hfb = spool.tile([BL, NB], F32, tag="hfb")
                    nc.scalar.activation(hfb[:], psb[:], Tanh)
                    nc.sync.dma_start(hfb_d[:], hfb[:])

                # ---- logits every CHUNK steps ----
                if slot == CHUNK - 1:
                    c = t // CHUNK
                    pl = psl_pool.tile([V, SQ], F32)
                    for k in range(HB):
                        nc.tensor.matmul(
                            pl[:], who_sb[:, k * V:(k + 1) * V],
                            hist3[par][:, k, :],
                            start=(k == 0), stop=(k == HB - 1),
                        )
                    lt = spool.tile([V, SQ], F32, tag="lt")
                    nc.vector.tensor_scalar_add(lt[:], pl[:], bo_sb[:])
                    nc.sync.dma_start(log_d[:, c * SQ:(c + 1) * SQ], lt[:])
            nc.sync.dma_start(hfa_d[:], hfa_sb[:])

    _split_excess_waits(nc)
    return nc


_NC_CACHE = {}


def kernel(x, embedding, W_ih, W_hh, b_h, W_ho, b_o):
    x = np.asarray(x)
    embedding = np.asarray(embedding, dtype=np.float32)
    W_ih = np.asarray(W_ih, dtype=np.float32)
    W_hh = np.asarray(W_hh, dtype=np.float32)
    b_h = np.asarray(b_h, dtype=np.float32)
    W_ho = np.asarray(W_ho, dtype=np.float32)
    b_o = np.asarray(b_o, dtype=np.float32)

    B, L = x.shape
    BL = B // N_CORES
    NB = (HB - SPLIT) * 128
    assert B % N_CORES == 0 and L % CHUNK == 0

    key = (L, BL)
    if key not in _NC_CACHE:
        _NC_CACHE[key] = _build_nc(L, BL)
    nc = _NC_CACHE[key]

    # ---- host-side input prep (layout / dtype lowering only) ----
    bf16 = ml_dtypes.bfloat16
    # group-B H-column device order: jj = q*128 + k'*32 + s holds
    # original group-B offset k'*128 + q*32 + s (q-major for the DVE
    # 32x32 transpose: one contiguous 128-col slice per partition
    # sub-range)
    qq, kk, ss = np.meshgrid(np.arange(4), np.arange((HB - SPLIT)),
                             np.arange(32), indexing="ij")
    perm = (kk * 128 + qq * 32 + ss).reshape(NB)
    # weight-stationary W_hh block (m,k) as lhsT [k-rows, m-cols]
    ws_arr = np.ascontiguousarray(
        W_hh.reshape(HB, 128, HB, 128)[:, :, :SPLIT, :]
        .transpose(1, 2, 0, 3).reshape(128, SPLIT * HB * 128).astype(bf16))
    # moving-operand W_hh rows, k-block on partitions: [p, k*NB+jj]
    wm_arr = np.ascontiguousarray(
        W_hh.reshape(HB, 128, H)[:, :, SPLIT * 128 + perm]
        .transpose(1, 0, 2).reshape(128, HB * NB).astype(bf16))
    # input-projection lookup table (group-B columns permuted to match)
    table_f = embedding @ W_ih + b_h
    table_f = np.concatenate(
        [table_f[:, :SPLIT * 128], table_f[:, SPLIT * 128 + perm]], axis=1)
    table = np.ascontiguousarray(table_f.astype(bf16))
    # W_ho block k = W_ho[128k:128k+128, :] at cols k*V
    who_arr = np.ascontiguousarray(
        W_ho.reshape(HB, 128, V).transpose(1, 0, 2).reshape(128, HB * V)
        .astype(bf16))
    bo_arr = np.ascontiguousarray(b_o.reshape(V, 1).astype(np.float32))

    iota = np.arange(V, dtype=x.dtype)
    in_maps = []
    for c in range(N_CORES):
        xc = x[c * BL:(c + 1) * BL, :]              # [BL, L]
        oh = (iota[:, None, None] == xc.T[None, :, :]).astype(bf16)
        in_maps.append({
            "w_stat": ws_arr,
            "w_mov": wm_arr,
            "table": table,
            "onehot": np.ascontiguousarray(oh.reshape(V, L * BL)),
            "w_ho": who_arr,
            "b_o": bo_arr,
        })

    res = run_bass_kernel_spmd(
        nc, in_maps, core_ids=list(range(N_CORES)), trace=TRACE)
    LAST_RESULT["exec_time_ns"] = res.exec_time_ns
    LAST_RESULT["mean_exec_time_ns"] = res.mean_exec_time_ns
    LAST_RESULT["instructions_and_trace"] = res.instructions_and_trace

    logits = np.empty((B, L, V), dtype=np.float32)
    final_hidden = np.empty((B, H), dtype=np.float32)
    for c in range(N_CORES):
        r = res.results[c]
        # logits_t[v, t*BL+b] -> [b, t, v]
        logits[c * BL:(c + 1) * BL] = (
            r["logits_t"].reshape(V, L, BL).transpose(2, 1, 0))
        # h_final_a[p, m*BL+b] -> [b, m*128+p];  h_final_b already [b, j]
        fh = final_hidden[c * BL:(c + 1) * BL]
        fh[:, :SPLIT * 128] = (
            r["h_final_a"].reshape(128, SPLIT, BL).transpose(2, 1, 0)
            .reshape(BL, SPLIT * 128))
        fh[:, SPLIT * 128 + perm] = r["h_final_b"]
    return logits, final_hidden


# revision 27
# speedup vs baseline: 1.0005x; 1.0005x over previous
"""CharRNN Trainium2 kernel: logits, final_hidden = rnn(x, embedding, ...).

Strategy (8 NeuronCores, data-parallel over batch, 32 rows/core):
  - Host folds embedding @ W_ih + b_h into a [VOCAB, HIDDEN] table and
    lowers x to a one-hot indicator; the input projection xp_t enters
    each step's PSUM accumulation via a one-hot matmul (start=True).
  - Hidden state lives transposed [H, B] in SBUF (bf16).
  - Port-split recurrence: the PE has independent SBUF read ports for
    the stationary (weights) and moving operands. H_out blocks
    m < SPLIT are computed weight-stationary (W_hh blocks via the
    FWL weight port, h^T moving, N=32); blocks m >= SPLIT are computed
    h-stationary (h^T chunks on the weight port, W_hh rows streamed
    512-wide on the moving port). The two streams share the array but
    load W through different ports, beating the single-port W-movement
    floor. The h-stationary half also keeps the PE HAM-warm (dense
    512-col matmuls at 2.4 GHz).
  - The h-stationary half lands batch-major; DVE 32x32 StreamTranspose
    puts it back into h^T form (PE untouched).
  - Every 16 steps the banked h^T history feeds the W_ho projection
    (N=512 matmuls), bias added on DVE, result DMA'd out transposed.
"""

import numpy as np
import ml_dtypes

import concourse.bass as bass
import concourse.mybir as mybir
import concourse.tile as tile
from concourse.bass_utils import run_bass_kernel_spmd

N_CORES = 8
V = 64        # vocab
E = 256       # embed
H = 1024      # hidden
HB = H // 128  # 8 H-blocks
CHUNK = 16    # steps per logits flush (16*32 = 512 moving cols)
SPLIT = 4     # H-blocks computed weight-stationary; rest h-stationary

BF16 = mybir.dt.bfloat16
F32 = mybir.dt.float32

TRACE = False           # set True from test harness to get NTFF profile
LAST_RESULT = {}        # exec_time_ns etc. stashed here for the harness


def _split_excess_waits(nc):
    """This walrus build encodes at most 1 sync wait per instruction
    (2 for EventSemaphore). Tile's tail drain can carry more; hoist the
    excess into preceding NOPs on the same engine."""
    ctr = 0
    for f in nc.m.functions:
        for bb in f.blocks:
            new_list = []
            changed = False
            for inst in bb.instructions:
                si = inst.sync_info
                cap = 2 if isinstance(inst, mybir.InstEventSemaphore) else 1
                if si is not None and si.on_wait and len(si.on_wait) > cap:
                    waits = list(si.on_wait)
                    keep, excess = waits[:cap], waits[cap:]
                    for w in excess:
                        new_list.append(mybir.InstNoOp(
                            name=f"wait_split_{ctr}",
                            engine=inst.engine,
                            bass_nofuse=True,
                            sync_info=mybir.SyncInfo(on_wait=[w], on_update=[]),
                        ))
                        ctr += 1
                    si.on_wait = keep
                    inst.sync_info = si
                    changed = True
                new_list.append(inst)
            if changed:
                bb.instructions = new_list
    return ctr


def _build_nc(L, BL):
    NB = (HB - SPLIT) * 128      # h-stationary H_out columns (<= 512)
    assert NB <= 512
    nc = bass.Bass()
    ws_d = nc.dram_tensor("w_stat", [128, SPLIT * HB * 128], BF16,
                          kind="ExternalInput")
    wm_d = nc.dram_tensor("w_mov", [128, HB * NB], BF16, kind="ExternalInput")
    tab_d = nc.dram_tensor("table", [V, H], BF16, kind="ExternalInput")
    oh_d = nc.dram_tensor("onehot", [V, L * BL], BF16, kind="ExternalInput")
    who_d = nc.dram_tensor("w_ho", [128, HB * V], BF16, kind="ExternalInput")
    bo_d = nc.dram_tensor("b_o", [V, 1], F32, kind="ExternalInput")
    log_d = nc.dram_tensor("logits_t", [V, L * BL], F32, kind="ExternalOutput")
    hfa_d = nc.dram_tensor("h_final_a", [128, SPLIT * BL], F32,
                           kind="ExternalOutput")
    hfb_d = nc.dram_tensor("h_final_b", [BL, NB], F32, kind="ExternalOutput")

    Tanh = mybir.ActivationFunctionType.Tanh
    SQ = CHUNK * BL          # 512: cols per hist k-block

    with tile.TileContext(nc) as tc:
        with (
            tc.tile_pool(name="const", bufs=1) as cpool,
            tc.tile_pool(name="hist", bufs=1) as hpool,
            tc.tile_pool(name="stage", bufs=3) as spool,
            tc.tile_pool(name="hb", bufs=2) as hbpool,
            tc.tile_pool(name="psa", bufs=4, space="PSUM") as psa_pool,
            tc.tile_pool(name="psb", bufs=2, space="PSUM") as psb_pool,
            tc.tile_pool(name="psl", bufs=2, space="PSUM") as psl_pool,
        ):
            ws_sb = cpool.tile([128, SPLIT * HB * 128], BF16, tag="ws")
            wm_sb = cpool.tile([128, HB * NB], BF16, tag="wm")
            tab_sb = cpool.tile([V, H], BF16, tag="tab")
            oh_sb = cpool.tile([V, L * BL], BF16, tag="oh")
            who_sb = cpool.tile([128, HB * V], BF16, tag="who")
            bo_sb = cpool.tile([V, 1], F32, tag="bo")
            hfa_sb = cpool.tile([128, SPLIT * BL], F32, tag="hfa")
            # order matters: step 0 gates only on the table and the first
            # few steps' onehot columns; the 2MB weight loads (not needed
            # until t=1) and the onehot tail go behind them
            OH0 = min(32, L) * BL
            nc.sync.dma_start(tab_sb[:], tab_d[:])
            nc.sync.dma_start(oh_sb[:, :OH0], oh_d[:, :OH0])
            nc.sync.dma_start(bo_sb[:], bo_d[:])
            nc.sync.dma_start(ws_sb[:], ws_d[:])
            nc.sync.dma_start(wm_sb[:], wm_d[:])
            if OH0 < L * BL:
                nc.sync.dma_start(oh_sb[:, OH0:], oh_d[:, OH0:])
            nc.sync.dma_start(who_sb[:], who_d[:])

            # h^T history, double-buffered by 16-step chunk parity:
            # hist[par][:, k*SQ + slot*BL + b] = h_t[k*128+p, b], slot=t%16
            hist = [hpool.tile([128, HB * SQ], BF16, tag=f"h{p}",
                               name=f"hist{p}") for p in range(2)]
            hist3 = [h.rearrange("p (k s) -> p k s", k=HB) for h in hist]

            for t in range(L):
                par, slot = (t // CHUNK) % 2, t % CHUNK
                ppar, pslot = ((t - 1) // CHUNK) % 2, (t - 1) % CHUNK
                oh_t = oh_sb[:, t * BL:(t + 1) * BL]

                # ---- group B: h-stationary, W streamed on moving port ----
                psb = psb_pool.tile([BL, NB], F32)
                # xp (batch-major): onehot_t stationary, table slice moving
                nc.tensor.matmul(
                    psb[:], oh_t, tab_sb[:, SPLIT * 128:],
                    start=True, stop=(t == 0))
                if t > 0:
                    for k in range(HB):
                        nc.tensor.matmul(
                            psb[:],
                            hist3[ppar][:, k, pslot * BL:(pslot + 1) * BL],
                            wm_sb[:, k * NB:(k + 1) * NB],
                            start=False, stop=(k == HB - 1),
                        )

                # ---- group A: weight-stationary W blocks, h^T moving ----
                # per-pair PSUM tiles (separate banks) so the early tanh of
                # one pair never bank-conflicts with matmuls of the next
                psa_tiles = []
                for pr in range(SPLIT // 2):
                    psa = psa_pool.tile([128, 2 * BL], F32)
                    psa_tiles.append(psa)
                    for mm in range(2):
                        m = 2 * pr + mm
                        reg = psa[:, mm * BL:(mm + 1) * BL]
                        nc.tensor.matmul(
                            reg, tab_sb[:, m * 128:(m + 1) * 128], oh_t,
                            start=True, stop=(t == 0))
                        if t > 0:
                            for k in range(HB):
                                nc.tensor.matmul(
                                    reg,
                                    ws_sb[:, (m * HB + k) * 128:(m * HB + k + 1) * 128],
                                    hist3[ppar][:, k, pslot * BL:(pslot + 1) * BL],
                                    start=False, stop=(k == HB - 1),
                                )
                    # early tanh per pair of A-blocks so next step's group B
                    # (k-order 0..7) finds its stationaries ready
                    nc.scalar.activation(
                        hist3[par][:, 2 * pr:2 * pr + 2, slot * BL:(slot + 1) * BL],
                        psa.rearrange("p (k s) -> p k s", k=2),
                        Tanh)

                # ---- group B epilogue: tanh + DVE 32x32 transpose ----
                # hb columns are host-permuted q-major: jj = q*128+k'*32+s
                # holds H column SPLIT*128 + k'*128 + q*32 + s, so each
                # contiguous 128-col slice transposes into one partition
                # sub-range across all 4 k-blocks (2D-in + 3D-out form).
                hb = hbpool.tile([BL, NB], BF16, tag="hb")
                nc.scalar.activation(hb[:], psb[:], Tanh)
                for q in range(4):
                    nc.vector.transpose(
                        hist3[par][q * 32:(q + 1) * 32, SPLIT:, slot * BL:(slot + 1) * BL],
                        hb[:, q * 128:(q + 1) * 128],
                    )

                if t == L - 1:
                    for pr in range(SPLIT // 2):
                        nc.scalar.activation(
                            hfa_sb[:, 2 * pr * BL:(2 * pr + 2) * BL],
                            psa_tiles[pr][:], Tanh)
                    hfb = spool.tile([BL, NB], F32, tag="hfb")
                    nc.scalar.activation(hfb[:], psb[:], Tanh)
                    nc.sync.dma_start(hfb_d[:], hfb[:])

                # ---- logits every CHUNK steps ----
                if slot == CHUNK - 1:
                    c = t // CHUNK
                    pl = psl_pool.tile([V, SQ], F32)
                    for k in range(HB):
                        nc.tensor.matmul(
                            pl[:], who_sb[:, k * V:(k + 1) * V],
                            hist3[par][:, k, :],
                            start=(k == 0), stop=(k == HB - 1),
                        )
                    lt = spool.tile([V, SQ], F32, tag="lt")
                    nc.vector.tensor_scalar_add(lt[:], pl[:], bo_sb[:])
                    nc.sync.dma_start(log_d[:, c * SQ:(c + 1) * SQ], lt[:])
            nc.sync.dma_start(hfa_d[:], hfa_sb[:])

    _split_excess_waits(nc)
    return nc


_NC_CACHE = {}


def kernel(x, embedding, W_ih, W_hh, b_h, W_ho, b_o):
    x = np.asarray(x)
    embedding = np.asarray(embedding, dtype=np.float32)
    W_ih = np.asarray(W_ih, dtype=np.float32)
    W_hh = np.asarray(W_hh, dtype=np.float32)
    b_h = np.asarray(b_h, dtype=np.float32)
    W_ho = np.asarray(W_ho, dtype=np.float32)
    b_o = np.asarray(b_o, dtype=np.float32)

    B, L = x.shape
    BL = B // N_CORES
    NB = (HB - SPLIT) * 128
    assert B % N_CORES == 0 and L % CHUNK == 0

    key = (L, BL)
    if key not in _NC_CACHE:
        _NC_CACHE[key] = _build_nc(L, BL)
    nc = _NC_CACHE[key]

    # ---- host-side input prep (layout / dtype lowering only) ----
    bf16 = ml_dtypes.bfloat16
    # group-B H-column device order: jj = q*128 + k'*32 + s holds
    # original group-B offset k'*128 + q*32 + s (q-major for the DVE
    # 32x32 transpose: one contiguous 128-col slice per partition
    # sub-range)
    qq, kk, ss = np.meshgrid(np.arange(4), np.arange((HB - SPLIT)),
                             np.arange(32), indexing="ij")
    perm = (kk * 128 + qq * 32 + ss).reshape(NB)
    # weight-stationary W_hh block (m,k) as lhsT [k-rows, m-cols]
    ws_arr = np.ascontiguousarray(
        W_hh.reshape(HB, 128, HB, 128)[:, :, :SPLIT, :]
        .transpose(1, 2, 0, 3).reshape(128, SPLIT * HB * 128).astype(bf16))
    # moving-operand W_hh rows, k-block on partitions: [p, k*NB+jj]
    wm_arr = np.ascontiguousarray(
        W_hh.reshape(HB, 128, H)[:, :, SPLIT * 128 + perm]
        .transpose(1, 0, 2).reshape(128, HB * NB).astype(bf16))
    # input-projection lookup table (group-B columns permuted to match)
    table_f = embedding @ W_ih + b_h
    table_f = np.concatenate(
        [table_f[:, :SPLIT * 128], table_f[:, SPLIT * 128 + perm]], axis=1)
    table = np.ascontiguousarray(table_f.astype(bf16))
    # W_ho block k = W_ho[128k:128k+128, :] at cols k*V
    who_arr = np.ascontiguousarray(
        W_ho.reshape(HB, 128, V).transpose(1, 0, 2).reshape(128, HB * V)
        .astype(bf16))
    bo_arr = np.ascontiguousarray(b_o.reshape(V, 1).astype(np.float32))

    iota = np.arange(V, dtype=x.dtype)
    in_maps = []
    for c in range(N_CORES):
        xc = x[c * BL:(c + 1) * BL, :]              # [BL, L]
        oh = (iota[:, None, None] == xc.T[None, :, :]).astype(bf16)
        in_maps.append({
            "w_stat": ws_arr,
            "w_mov": wm_arr,
            "table": table,
            "onehot": np.ascontiguousarray(oh.reshape(V, L * BL)),
            "w_ho": who_arr,
            "b_o": bo_arr,
        })

    res = run_bass_kernel_spmd(
        nc, in_maps, core_ids=list(range(N_CORES)), trace=TRACE)
    LAST_RESULT["exec_time_ns"] = res.exec_time_ns
    LAST_RESULT["mean_exec_time_ns"] = res.mean_exec_time_ns
    LAST_RESULT["instructions_and_trace"] = res.instructions_and_trace

    logits = np.empty((B, L, V), dtype=np.float32)
    final_hidden = np.empty((B, H), dtype=np.float32)
    for c in range(N_CORES):
        r = res.results[c]
        # logits_t[v, t*BL+b] -> [b, t, v]
        logits[c * BL:(c + 1) * BL] = (
            r["logits_t"].reshape(V, L, BL).transpose(2, 1, 0))
        # h_final_a[p, m*BL+b] -> [b, m*128+p];  h_final_b already [b, j]
        fh = final_hidden[c * BL:(c + 1) * BL]
        fh[:, :SPLIT * 128] = (
            r["h_final_a"].reshape(128, SPLIT, BL).transpose(2, 1, 0)
            .reshape(BL, SPLIT * 128))
        fh[:, SPLIT * 128 + perm] = r["h_final_b"]
    return logits, final_hidden
